# revision 3
# baseline (speedup 1.0000x reference)
"""v2: carry-chain-free Aaren scan kernel (fp16 DMA, bf16 matmul operands).

Layouts (per (b,h) pair, CH=128, nch=32, no padding):
- q,k packed TRANSPOSED [128 d, 4096 t] fp16. prod = q*k (one DVE 4x mul);
  alpha chunk c = ones-matmul with stationary prod[:, chunk] and moving
  ones[128,1] -> psA[:, c] COLUMN (t-major [128, 32], quadrant-legal,
  ap_size=1 so nearly free on PE).
- v,y packed t-major [128 s, 32c*128] fp16.
- wv staging [128, 32*129] bf16: per chunk 128 scaled v-cols + col 128 =
  W column (denominator rides through all matmuls as a 129th column).

Math: running chunk max M_c stabilizer; W = exp(Ap - M) (fp32 for scale
ops, bf16 copy for matmuls; bf16 keeps e^-40-scale weights from flushing).
Cross-chunk state in closed form (no serial carry chain):
  S_c (values) = ones-matmul with stationary wv chunk -> psS_T[:, c] column;
  transposed once per pair to c-major; S^d col = ones-matmul of W_t.
  C = G' @ S_sb, G'[j,c] = exp(min(M_j-M_c,0)) * [j<c] (mask after exp).
  C rows flattened to partition-0 row C_row via one tiny SBUF->SBUF DMA
  (matmul moving operands must be quadrant-aligned).
Per chunk: psY = U2^T wv_c + rank-1 ones x C_row-slice; y = psY[:,0:128]
* R[:,c] with R = 1/(U2^T W_t + ones x CdT) once per pair.

Sharding: B*H = 64 pairs -> 8 pairs per core, no cross-core traffic.
"""

import sys

if "/opt/trn_rl_repo" not in sys.path:
    sys.path.insert(0, "/opt/trn_rl_repo")

import numpy as np

B, H, N, D = 4, 16, 4096, 128
NCORES = 8
PAIRS = B * H // NCORES  # 8 (b,h) pairs per core

CH = 128
NCH = N // CH       # 32 chunks
FW = N
WVW = NCH * (CH + 1)
NEG = -3.0e38
STORE_LAG = 3
LOOKAHEAD = 1
ALPHA_MODE = "hybrid"


def build_nc(pairs=PAIRS, n=N, order=0, evac_dve=4, presc_pool=24,
             py_bufs=4, pg_bufs=3, skip_carry=False, skip_alpha=False,
             qk_bufs=4, wv_bufs=3, sc_bufs=4, sp_bufs=3):
    import concourse.tile as tile
    from concourse import bacc, mybir
    from contextlib import ExitStack

    f32 = mybir.dt.float32
    f16 = mybir.dt.float16
    bf16 = mybir.dt.bfloat16
    Alu = mybir.AluOpType
    Act = mybir.ActivationFunctionType
    X = mybir.AxisListType.X

    nch = n // CH
    fw = n
    wvw = nch * (CH + 1)

    nc = bacc.Bacc("TRN2", target_bir_lowering=False, debug=False)

    qd = nc.dram_tensor("q", [pairs, 128, fw], f16, kind="ExternalInput")
    kd = nc.dram_tensor("k", [pairs, 128, fw], f16, kind="ExternalInput")
    vd = nc.dram_tensor("v", [pairs, 128, fw], f16, kind="ExternalInput")
    yd = nc.dram_tensor("y", [pairs, 128, fw], f16, kind="ExternalOutput")

    with tile.TileContext(nc) as tc, ExitStack() as ctx:
        cpool = ctx.enter_context(tc.tile_pool(name="consts", bufs=1))
        spool = ctx.enter_context(tc.tile_pool(name="stats", bufs=sp_bufs))
        wpool = ctx.enter_context(tc.tile_pool(name="wts", bufs=4))
        rpool = ctx.enter_context(tc.tile_pool(name="rcps", bufs=3))
        qkpool = ctx.enter_context(tc.tile_pool(name="qk", bufs=qk_bufs))
        scpool = ctx.enter_context(tc.tile_pool(name="scr", bufs=sc_bufs))
        vpool = ctx.enter_context(tc.tile_pool(name="vv", bufs=8))
        wvpool = ctx.enter_context(tc.tile_pool(name="wv", bufs=wv_bufs))
        pypool = ctx.enter_context(tc.tile_pool(name="py", bufs=py_bufs, space="PSUM"))
        chpool = ctx.enter_context(tc.tile_pool(name="ch", bufs=pg_bufs, space="PSUM"))
        pbpool = ctx.enter_context(tc.tile_pool(name="pb", bufs=1, space="PSUM"))

        # ---- constants -------------------------------------------------
        iota_f = cpool.tile([128, 128], f32, tag="iotaf")
        nc.gpsimd.iota(iota_f[:], [[1, 128]], channel_multiplier=0,
                       allow_small_or_imprecise_dtypes=True)
        iota_p = cpool.tile([128, 1], f32, tag="iotap")
        nc.gpsimd.iota(iota_p[:], [[0, 1]], channel_multiplier=1,
                       allow_small_or_imprecise_dtypes=True)
        u2f = cpool.tile([128, 128], f32, tag="u2f")
        nc.vector.tensor_scalar(u2f[:], iota_f[:], iota_p[:], None, Alu.is_ge)
        u2h = cpool.tile([128, 128], bf16, tag="u2h")
        nc.vector.tensor_copy(u2h[:], u2f[:])
        ident = cpool.tile([128, 128], f32, tag="ident")
        nc.vector.tensor_scalar(ident[:], iota_f[:], iota_p[:], None,
                                Alu.is_equal)
        identb = cpool.tile([128, 128], bf16, tag="identb")
        nc.vector.tensor_copy(identb[:], ident[:])
        mltf = cpool.tile([32, 32], f32, tag="mltf")
        nc.vector.tensor_scalar(mltf[:], iota_f[0:32, 0:32], iota_p[0:32, :],
                                None, Alu.is_gt)
        mltb = cpool.tile([32, 32], bf16, tag="mltb")
        nc.vector.tensor_copy(mltb[:], mltf[:])
        ones_col_h = cpool.tile([128, 1], f16, tag="onescolh")
        nc.gpsimd.memset(ones_col_h[:], 1.0)
        ones_col_b = cpool.tile([128, 1], bf16, tag="onescolb")
        nc.gpsimd.memset(ones_col_b[:], 1.0)
        ones_row32 = cpool.tile([1, 32], f32, tag="onesrow32")
        nc.gpsimd.memset(ones_row32[:], 1.0)
        ones_row_f = cpool.tile([1, 128], f32, tag="onesrowf")
        nc.gpsimd.memset(ones_row_f[:], 1.0)
        ones_row_b = cpool.tile([1, 128], bf16, tag="onesrowb")
        nc.gpsimd.memset(ones_row_b[:], 1.0)

        def emit_loads(p):
            qall = qkpool.tile([128, fw], f16, tag="qa")
            kall = qkpool.tile([128, fw], f16, tag="ka")
            nc.sync.dma_start(qall[:], qd[p])
            nc.sync.dma_start(kall[:], kd[p])
            vall = vpool.tile([128, fw], f16, tag="va", name=f"va{p}")
            nc.sync.dma_start(vall[:], vd[p])
            return qall, kall, vall

        def emit_head(p, qall, kall, vall):

            # ---- alpha via fused q*k multiply-accumulate (DVE) ---------
            # scalar_tensor_tensor: out = (q bypass 1) * k, accum_out = per-
            # partition free-dim sum = alpha column for the chunk.
            Ap = spool.tile([128, nch], f32, tag="Ap")
            for c in range(0 if skip_alpha else nch):
                scr = scpool.tile([128, D], f16, tag="scr")
                nc.vector.scalar_tensor_tensor(
                    scr[:], qall[:, c * CH:(c + 1) * CH], 1.0,
                    kall[:, c * CH:(c + 1) * CH],
                    op0=Alu.bypass, op1=Alu.mult,
                    accum_out=Ap[:, c:c + 1])

            # ---- running chunk max M (regions of one psum bank tile) ---
            chain = chpool.tile([128, 512], f32, tag="ch", name=f"chain{p}")
            psT2 = chain
            nc.tensor.transpose(psT2[0:32, 256:384], Ap[:], ident[:])
            mu = spool.tile([32, 1], f32, tag="mu")
            nc.vector.tensor_reduce(mu[:], psT2[0:32, 256:384],
                                    axis=X, op=Alu.max)
            psT = chain
            nc.tensor.transpose(psT[0:1, 448:480], mu[:], ident[0:32, 0:32])
            mrow = spool.tile([1, 32], f32, tag="mrow")
            nc.vector.tensor_copy(mrow[:], psT[0:1, 448:480])
            Mrow = spool.tile([1, 32], f32, tag="Mrow")
            nc.vector.tensor_tensor_scan(Mrow[:], mrow[:], mrow[:],
                                         initial=NEG, op0=Alu.max, op1=Alu.max)
            negM = spool.tile([1, 32], f32, tag="negM")
            nc.vector.tensor_scalar_mul(negM[:], Mrow[:], -1.0)

            # ---- G'[j,c] = exp(min(M_j - M_c, 0)) * [j < c] ------------
            psG = chain
            nc.tensor.matmul(psG[32:64, 384:416], Mrow[:], ones_row32[:],
                             start=True, stop=False)
            nc.tensor.matmul(psG[32:64, 384:416], ones_row32[:], negM[:],
                             start=False, stop=True)
            gclip = spool.tile([32, 32], f32, tag="gclip")
            nc.vector.tensor_scalar_min(gclip[:], psG[32:64, 384:416], 0.0)
            gexp = spool.tile([32, 32], bf16, tag="gexp")
            nc.scalar.activation(gexp[:], gclip[:], Act.Exp)
            Gh = spool.tile([32, 32], bf16, tag="Gh")
            nc.vector.tensor_mul(Gh[:], gexp[:], mltb[:])

            # ---- weights W = exp(Ap - M): f32 + bf16 copies ------------
            psB = chain
            nc.tensor.matmul(psB[0:128, 416:448], ones_row_f[:], Mrow[:],
                             start=True, stop=True)
            wsub = spool.tile([128, 32], f32, tag="wsub")
            nc.vector.tensor_sub(wsub[:], Ap[:], psB[0:128, 416:448])
            W_f = wpool.tile([128, 32], f32, tag="W_f")
            nc.scalar.activation(W_f[:], wsub[:], Act.Exp)
            return vall, Gh, W_f, chain

        def emit_head2(p, vall, Gh, W_f, chain):
            # ---- prescale v into wv (129-col blocks) -------------------
            wv = wvpool.tile([128, wvw], bf16, tag="wv")
            for c in range(nch):
                dst = wv[:, c * 129:c * 129 + 128]
                src = vall[:, c * CH:(c + 1) * CH]
                if (c * 32) // 32 % 32 < presc_pool if False else c % 32 < presc_pool:
                    nc.gpsimd.tensor_scalar_mul(dst, src, W_f[:, c:c + 1])
                else:
                    nc.scalar.activation(dst, src, Act.Copy,
                                         scale=W_f[:, c:c + 1])
            wv_d = wv[:].rearrange("p (c x) -> p c x", x=129)[:, :, 128:129]
            nc.vector.tensor_copy(
                wv_d, W_f[:].rearrange("p (c x) -> p c x", x=1))

            if skip_carry:
                Rp = rpool.tile([128, 32], f32, tag="Rp")
                nc.gpsimd.memset(Rp[:], 1.0)
                return wv, Rp, None, vall
            # ---- chunk sums S via PE column matmuls + transpose --------
            psS_T = chain
            for c in range(nch):
                nc.tensor.matmul(psS_T[0:128, c:c + 1],
                                 wv[:, c * 129:c * 129 + 128],
                                 ones_col_b[:], start=True, stop=True)
            psSd = psS_T
            nc.tensor.matmul(psSd[0:32, 32:33], wv_d.squeeze(2),
                             ones_col_b[:], start=True, stop=True)
            S_T_sb = spool.tile([128, 32], bf16, tag="S_T_sb")
            nc.scalar.copy(S_T_sb[:], psS_T[0:128, 0:32])
            psS2 = pbpool.tile([128, 129], bf16, tag="pb", name=f"psS2{p}")
            nc.tensor.transpose(psS2[0:32, 0:128], S_T_sb[:], identb[:])
            S_sb = spool.tile([32, 129], bf16, tag="S_sb")
            nc.scalar.copy(S_sb[0:32, 0:128], psS2[0:32, 0:128])
            nc.scalar.copy(S_sb[0:32, 128:129], psSd[0:32, 32:33])

            # ---- carries C = G' @ S ------------------------------------
            psN = chain
            nc.tensor.matmul(psN[0:32, 64:193], Gh[:], S_sb[:],
                             start=True, stop=True)
            C_sb = spool.tile([32, 129], bf16, tag="C_sb")
            nc.scalar.copy(C_sb[:], psN[0:32, 64:193])
            # Flatten carries to a partition-0 row (rank-1 carry matmuls need
            # quadrant-aligned moving operands).
            C_row = spool.tile([1, nch * 129], bf16, tag="C_row")
            nc.sync.dma_start(C_row[:], C_sb[:])

            # ---- denominators (reciprocal deferred to the body) --------
            CdT = spool.tile([1, 32], bf16, tag="CdT")
            nc.vector.tensor_copy(
                CdT[:],
                C_row[:].rearrange("p (c x) -> p c x", x=129)[:, :, 128])
            psD = chain
            nc.tensor.matmul(psD[0:128, 200:232], u2h[:], wv_d.squeeze(2),
                             start=True, stop=False)
            nc.tensor.matmul(psD[0:128, 200:232], ones_row_b[:], CdT[:],
                             start=False, stop=True)
            Rp = rpool.tile([128, 32], f32, tag="Rp")
            nc.vector.reciprocal(Rp[:], psD[0:128, 200:232])
            return wv, Rp, C_row, vall

        def emit_body(p, ctx2):
            wv, Rp, C_row, vall = ctx2
            # ---- main chunk matmuls (4 chunks per matmul; moving operand
            # skips the d-cols via a strided AP) + multi-chunk evacs ------
            # y is written in place over v (v is dead after the prescale)
            yall = vall
            for c0 in range(0, nch, 4):
                psY = pypool.tile([128, 512], f32, tag="psY")
                mv = wv[:].rearrange("p (c x) -> p c x",
                                     x=129)[:, c0:c0 + 4, 0:128]
                # tri part depends only on wv (can run early); the rank-1
                # carry gates on C_row, so only it + the evacs wait on C
                nc.tensor.matmul(psY[:], u2h[:], mv, start=True,
                                 stop=C_row is None)
                if C_row is not None:
                    cmv = C_row[:].rearrange(
                        "p (c x) -> p c x", x=129)[:, c0:c0 + 4, 0:128]
                    nc.tensor.matmul(psY[:], ones_row_b[:], cmv,
                                     start=False, stop=True)
                m = (c0 // 4 + p) % 8
                if m < evac_dve:
                    # one DVE op for all 4 chunks: y = psY * R (R broadcast
                    # along the value columns with a stride-0 AP)
                    rb = Rp[:, c0:c0 + 4].unsqueeze(2).broadcast_to(
                        [128, 4, CH])
                    nc.vector.tensor_mul(
                        yall[:, c0 * CH:(c0 + 4) * CH].rearrange(
                            "p (c d) -> p c d", d=CH),
                        psY[:].rearrange("p (c d) -> p c d", d=CH), rb)
                else:
                    for i in range(4):
                        c = c0 + i
                        nc.scalar.activation(yall[:, c * CH:(c + 1) * CH],
                                             psY[:, i * CH:(i + 1) * CH],
                                             Act.Copy, scale=Rp[:, c:c + 1])
            nc.sync.dma_start(yd[p], yall[:])

        # all loads up front (y reuses v tiles, so everything fits),
        # then a 3-stage software pipeline in readiness order:
        # head1(p) | body(p-2) | head2(p-1)
        loads = [emit_loads(p) for p in range(pairs)]
        h1q, h2q = [], []
        def do_h1(p):
            if p < pairs:
                h1q.append((p, emit_head(p, *loads[p])))
        def do_h2():
            if h1q:
                p0, c0 = h1q.pop(0)
                h2q.append((p0, emit_head2(p0, *c0)))
        def do_b():
            if h2q:
                emit_body(*h2q.pop(0))
        orders = {
            0: lambda p: (do_h1(p), do_b(), do_h2()),
            1: lambda p: (do_b(), do_h2(), do_h1(p)),
            2: lambda p: (do_h1(p), do_h2(), do_b()),
            3: lambda p: (do_b(), do_h1(p), do_h2()),
            4: lambda p: (do_h2(), do_b(), do_h1(p)),
            5: lambda p: (do_h2(), do_h1(p), do_b()),
        }
        for p in range(pairs + 2):
            orders[order](p)

    nc.compile()
    return nc


def pack_qk(x):
    """[m, N, D] -> [m, D, N] fp16 (transposed)."""
    return np.ascontiguousarray(np.asarray(x, np.float16).transpose(0, 2, 1))


def pack_t(x):
    """[m, N, D] -> [m, 128 s, nch*128] fp16 t-major chunks."""
    m = x.shape[0]
    xp = np.asarray(x, np.float16).reshape(m, NCH, CH, D)
    return np.ascontiguousarray(
        xp.transpose(0, 2, 1, 3).reshape(m, CH, NCH * D))


def unpack_y(yp):
    """[m, 128, nch*128] -> [m, N, D] fp32."""
    m = yp.shape[0]
    yv = yp.reshape(m, CH, NCH, D).transpose(0, 2, 1, 3)
    return np.ascontiguousarray(yv.reshape(m, N, D).astype(np.float32))


_cached = {}


def _get_nc():
    if "nc" not in _cached:
        _cached["nc"] = build_nc()
    return _cached["nc"]


def run_on_hw(q, k, v, trace=False):
    """q,k,v: np [B,H,N,D] f32 -> (y [B,H,N,D], exec_time_ns or None)."""
    from concourse.bass_utils import run_bass_kernel_spmd

    nc = _get_nc()
    qp = pack_t(np.asarray(q, np.float32).reshape(B * H, N, D))
    kp = pack_t(np.asarray(k, np.float32).reshape(B * H, N, D))
    vp = pack_t(np.asarray(v, np.float32).reshape(B * H, N, D))
    in_maps = [
        {
            "q": qp[c * PAIRS:(c + 1) * PAIRS],
            "k": kp[c * PAIRS:(c + 1) * PAIRS],
            "v": vp[c * PAIRS:(c + 1) * PAIRS],
        }
        for c in range(NCORES)
    ]
    res = run_bass_kernel_spmd(nc, in_maps, list(range(NCORES)), trace=False)
    yp = np.concatenate([np.asarray(res.results[c]["y"]) for c in range(NCORES)],
                        axis=0)
    return unpack_y(yp).reshape(B, H, N, D), res.exec_time_ns


def kernel(q, k, v):
    y, _ = run_on_hw(q, k, v, trace=False)
    return y


# revision 4
# speedup vs baseline: 1.0152x; 1.0152x over previous
"""v2: carry-chain-free Aaren scan kernel (fp16 DMA, bf16 matmul operands).

Layouts (per (b,h) pair, CH=128, nch=32, no padding):
- q,k packed TRANSPOSED [128 d, 4096 t] fp16. prod = q*k (one DVE 4x mul);
  alpha chunk c = ones-matmul with stationary prod[:, chunk] and moving
  ones[128,1] -> psA[:, c] COLUMN (t-major [128, 32], quadrant-legal,
  ap_size=1 so nearly free on PE).
- v,y packed t-major [128 s, 32c*128] fp16.
- wv staging [128, 32*129] bf16: per chunk 128 scaled v-cols + col 128 =
  W column (denominator rides through all matmuls as a 129th column).

Math: running chunk max M_c stabilizer; W = exp(Ap - M) (fp32 for scale
ops, bf16 copy for matmuls; bf16 keeps e^-40-scale weights from flushing).
Cross-chunk state in closed form (no serial carry chain):
  S_c (values) = ones-matmul with stationary wv chunk -> psS_T[:, c] column;
  transposed once per pair to c-major; S^d col = ones-matmul of W_t.
  C = G' @ S_sb, G'[j,c] = exp(min(M_j-M_c,0)) * [j<c] (mask after exp).
  C rows flattened to partition-0 row C_row via one tiny SBUF->SBUF DMA
  (matmul moving operands must be quadrant-aligned).
Per chunk: psY = U2^T wv_c + rank-1 ones x C_row-slice; y = psY[:,0:128]
* R[:,c] with R = 1/(U2^T W_t + ones x CdT) once per pair.

Sharding: B*H = 64 pairs -> 8 pairs per core, no cross-core traffic.
"""

import sys

if "/opt/trn_rl_repo" not in sys.path:
    sys.path.insert(0, "/opt/trn_rl_repo")

import numpy as np

B, H, N, D = 4, 16, 4096, 128
NCORES = 8
PAIRS = B * H // NCORES  # 8 (b,h) pairs per core

CH = 128
NCH = N // CH       # 32 chunks
FW = N
WVW = NCH * (CH + 1)
NEG = -3.0e38
STORE_LAG = 3
LOOKAHEAD = 1
ALPHA_MODE = "hybrid"


def build_nc(pairs=PAIRS, n=N, order=0, evac_dve=6, presc_pool=16,
             alpha_pool=0,
             py_bufs=4, pg_bufs=3, skip_carry=False, skip_alpha=False,
             qk_bufs=4, wv_bufs=3, sc_bufs=4, sp_bufs=3):
    import concourse.tile as tile
    from concourse import bacc, mybir
    from contextlib import ExitStack

    f32 = mybir.dt.float32
    f16 = mybir.dt.float16
    bf16 = mybir.dt.bfloat16
    Alu = mybir.AluOpType
    Act = mybir.ActivationFunctionType
    X = mybir.AxisListType.X

    nch = n // CH
    fw = n
    wvw = nch * (CH + 1)

    nc = bacc.Bacc("TRN2", target_bir_lowering=False, debug=False)

    qd = nc.dram_tensor("q", [pairs, 128, fw], f16, kind="ExternalInput")
    kd = nc.dram_tensor("k", [pairs, 128, fw], f16, kind="ExternalInput")
    vd = nc.dram_tensor("v", [pairs, 128, fw], f16, kind="ExternalInput")
    yd = nc.dram_tensor("y", [pairs, 128, fw], f16, kind="ExternalOutput")

    with tile.TileContext(nc) as tc, ExitStack() as ctx:
        cpool = ctx.enter_context(tc.tile_pool(name="consts", bufs=1))
        spool = ctx.enter_context(tc.tile_pool(name="stats", bufs=sp_bufs))
        wpool = ctx.enter_context(tc.tile_pool(name="wts", bufs=4))
        rpool = ctx.enter_context(tc.tile_pool(name="rcps", bufs=3))
        qkpool = ctx.enter_context(tc.tile_pool(name="qk", bufs=qk_bufs))
        scpool = ctx.enter_context(tc.tile_pool(name="scr", bufs=sc_bufs))
        vpool = ctx.enter_context(tc.tile_pool(name="vv", bufs=8))
        wvpool = ctx.enter_context(tc.tile_pool(name="wv", bufs=wv_bufs))
        pypool = ctx.enter_context(tc.tile_pool(name="py", bufs=py_bufs, space="PSUM"))
        chpool = ctx.enter_context(tc.tile_pool(name="ch", bufs=pg_bufs, space="PSUM"))
        pbpool = ctx.enter_context(tc.tile_pool(name="pb", bufs=1, space="PSUM"))

        # ---- constants -------------------------------------------------
        iota_f = cpool.tile([128, 128], f32, tag="iotaf")
        nc.gpsimd.iota(iota_f[:], [[1, 128]], channel_multiplier=0,
                       allow_small_or_imprecise_dtypes=True)
        iota_p = cpool.tile([128, 1], f32, tag="iotap")
        nc.gpsimd.iota(iota_p[:], [[0, 1]], channel_multiplier=1,
                       allow_small_or_imprecise_dtypes=True)
        u2f = cpool.tile([128, 128], f32, tag="u2f")
        nc.vector.tensor_scalar(u2f[:], iota_f[:], iota_p[:], None, Alu.is_ge)
        u2h = cpool.tile([128, 128], bf16, tag="u2h")
        nc.vector.tensor_copy(u2h[:], u2f[:])
        ident = cpool.tile([128, 128], f32, tag="ident")
        nc.vector.tensor_scalar(ident[:], iota_f[:], iota_p[:], None,
                                Alu.is_equal)
        identb = cpool.tile([128, 128], bf16, tag="identb")
        nc.vector.tensor_copy(identb[:], ident[:])
        mltf = cpool.tile([32, 32], f32, tag="mltf")
        nc.vector.tensor_scalar(mltf[:], iota_f[0:32, 0:32], iota_p[0:32, :],
                                None, Alu.is_gt)
        mltb = cpool.tile([32, 32], bf16, tag="mltb")
        nc.vector.tensor_copy(mltb[:], mltf[:])
        ones_col_h = cpool.tile([128, 1], f16, tag="onescolh")
        nc.gpsimd.memset(ones_col_h[:], 1.0)
        ones_col_b = cpool.tile([128, 1], bf16, tag="onescolb")
        nc.gpsimd.memset(ones_col_b[:], 1.0)
        ones_row32 = cpool.tile([1, 32], f32, tag="onesrow32")
        nc.gpsimd.memset(ones_row32[:], 1.0)
        ones_row_f = cpool.tile([1, 128], f32, tag="onesrowf")
        nc.gpsimd.memset(ones_row_f[:], 1.0)
        ones_row_b = cpool.tile([1, 128], bf16, tag="onesrowb")
        nc.gpsimd.memset(ones_row_b[:], 1.0)

        def emit_loads(p):
            qall = qkpool.tile([128, fw], f16, tag="qa")
            kall = qkpool.tile([128, fw], f16, tag="ka")
            nc.sync.dma_start(qall[:], qd[p])
            nc.sync.dma_start(kall[:], kd[p])
            vall = vpool.tile([128, fw], f16, tag="va", name=f"va{p}")
            nc.sync.dma_start(vall[:], vd[p])
            return qall, kall, vall

        def emit_head(p, qall, kall, vall):

            # ---- alpha via fused q*k multiply-accumulate (DVE) ---------
            # scalar_tensor_tensor: out = (q bypass 1) * k, accum_out = per-
            # partition free-dim sum = alpha column for the chunk.
            Ap = spool.tile([128, nch], f32, tag="Ap")
            for c in range(0 if skip_alpha else nch):
                scr = scpool.tile([128, D], f16, tag="scr")
                eng = nc.gpsimd if c % 8 < alpha_pool else nc.vector
                eng.scalar_tensor_tensor(
                    scr[:], qall[:, c * CH:(c + 1) * CH], 1.0,
                    kall[:, c * CH:(c + 1) * CH],
                    op0=Alu.bypass, op1=Alu.mult,
                    accum_out=Ap[:, c:c + 1])

            # ---- running chunk max M (regions of one psum bank tile) ---
            chain = chpool.tile([128, 512], f32, tag="ch", name=f"chain{p}")
            psT2 = chain
            nc.tensor.transpose(psT2[0:32, 256:384], Ap[:], ident[:])
            mu = spool.tile([32, 1], f32, tag="mu")
            nc.vector.tensor_reduce(mu[:], psT2[0:32, 256:384],
                                    axis=X, op=Alu.max)
            psT = chain
            nc.tensor.transpose(psT[0:1, 448:480], mu[:], ident[0:32, 0:32])
            mrow = spool.tile([1, 32], f32, tag="mrow")
            nc.vector.tensor_copy(mrow[:], psT[0:1, 448:480])
            Mrow = spool.tile([1, 32], f32, tag="Mrow")
            nc.vector.tensor_tensor_scan(Mrow[:], mrow[:], mrow[:],
                                         initial=NEG, op0=Alu.max, op1=Alu.max)
            negM = spool.tile([1, 32], f32, tag="negM")
            nc.vector.tensor_scalar_mul(negM[:], Mrow[:], -1.0)

            # ---- G'[j,c] = exp(min(M_j - M_c, 0)) * [j < c] ------------
            psG = chain
            nc.tensor.matmul(psG[32:64, 384:416], Mrow[:], ones_row32[:],
                             start=True, stop=False)
            nc.tensor.matmul(psG[32:64, 384:416], ones_row32[:], negM[:],
                             start=False, stop=True)
            gclip = spool.tile([32, 32], f32, tag="gclip")
            nc.vector.tensor_scalar_min(gclip[:], psG[32:64, 384:416], 0.0)
            gexp = spool.tile([32, 32], bf16, tag="gexp")
            nc.scalar.activation(gexp[:], gclip[:], Act.Exp)
            Gh = spool.tile([32, 32], bf16, tag="Gh")
            nc.vector.tensor_mul(Gh[:], gexp[:], mltb[:])

            # ---- weights W = exp(Ap - M): f32 + bf16 copies ------------
            psB = chain
            nc.tensor.matmul(psB[0:128, 416:448], ones_row_f[:], Mrow[:],
                             start=True, stop=True)
            wsub = spool.tile([128, 32], f32, tag="wsub")
            nc.vector.tensor_sub(wsub[:], Ap[:], psB[0:128, 416:448])
            W_f = wpool.tile([128, 32], f32, tag="W_f")
            nc.scalar.activation(W_f[:], wsub[:], Act.Exp)
            return vall, Gh, W_f, chain

        def emit_head2(p, vall, Gh, W_f, chain):
            # ---- prescale v into wv (129-col blocks) -------------------
            wv = wvpool.tile([128, wvw], bf16, tag="wv")
            for c in range(nch):
                dst = wv[:, c * 129:c * 129 + 128]
                src = vall[:, c * CH:(c + 1) * CH]
                if (c * 32) // 32 % 32 < presc_pool if False else c % 32 < presc_pool:
                    nc.gpsimd.tensor_scalar_mul(dst, src, W_f[:, c:c + 1])
                else:
                    nc.scalar.activation(dst, src, Act.Copy,
                                         scale=W_f[:, c:c + 1])
            wv_d = wv[:].rearrange("p (c x) -> p c x", x=129)[:, :, 128:129]
            nc.vector.tensor_copy(
                wv_d, W_f[:].rearrange("p (c x) -> p c x", x=1))

            if skip_carry:
                Rp = rpool.tile([128, 32], f32, tag="Rp")
                nc.gpsimd.memset(Rp[:], 1.0)
                return wv, Rp, None, vall
            # ---- chunk sums S via PE column matmuls + transpose --------
            psS_T = chain
            for c in range(nch):
                nc.tensor.matmul(psS_T[0:128, c:c + 1],
                                 wv[:, c * 129:c * 129 + 128],
                                 ones_col_b[:], start=True, stop=True)
            psSd = psS_T
            nc.tensor.matmul(psSd[0:32, 32:33], wv_d.squeeze(2),
                             ones_col_b[:], start=True, stop=True)
            S_T_sb = spool.tile([128, 32], bf16, tag="S_T_sb")
            nc.scalar.copy(S_T_sb[:], psS_T[0:128, 0:32])
            psS2 = pbpool.tile([128, 129], bf16, tag="pb", name=f"psS2{p}")
            nc.tensor.transpose(psS2[0:32, 0:128], S_T_sb[:], identb[:])
            S_sb = spool.tile([32, 129], bf16, tag="S_sb")
            nc.scalar.copy(S_sb[0:32, 0:128], psS2[0:32, 0:128])
            nc.scalar.copy(S_sb[0:32, 128:129], psSd[0:32, 32:33])

            # ---- carries C = G' @ S ------------------------------------
            psN = chain
            nc.tensor.matmul(psN[0:32, 64:193], Gh[:], S_sb[:],
                             start=True, stop=True)
            C_sb = spool.tile([32, 129], bf16, tag="C_sb")
            nc.scalar.copy(C_sb[:], psN[0:32, 64:193])
            # Flatten carries to a partition-0 row (rank-1 carry matmuls need
            # quadrant-aligned moving operands).
            C_row = spool.tile([1, nch * 129], bf16, tag="C_row")
            nc.sync.dma_start(C_row[:], C_sb[:])

            # ---- denominators (reciprocal deferred to the body) --------
            CdT = spool.tile([1, 32], bf16, tag="CdT")
            nc.vector.tensor_copy(
                CdT[:],
                C_row[:].rearrange("p (c x) -> p c x", x=129)[:, :, 128])
            psD = chain
            nc.tensor.matmul(psD[0:128, 200:232], u2h[:], wv_d.squeeze(2),
                             start=True, stop=False)
            nc.tensor.matmul(psD[0:128, 200:232], ones_row_b[:], CdT[:],
                             start=False, stop=True)
            Rp = rpool.tile([128, 32], f32, tag="Rp")
            nc.vector.reciprocal(Rp[:], psD[0:128, 200:232])
            return wv, Rp, C_row, vall

        def emit_body(p, ctx2):
            wv, Rp, C_row, vall = ctx2
            # ---- main chunk matmuls (4 chunks per matmul; moving operand
            # skips the d-cols via a strided AP) + multi-chunk evacs ------
            # y is written in place over v (v is dead after the prescale)
            yall = vall
            for c0 in range(0, nch, 4):
                psY = pypool.tile([128, 512], f32, tag="psY")
                mv = wv[:].rearrange("p (c x) -> p c x",
                                     x=129)[:, c0:c0 + 4, 0:128]
                # tri part depends only on wv (can run early); the rank-1
                # carry gates on C_row, so only it + the evacs wait on C
                nc.tensor.matmul(psY[:], u2h[:], mv, start=True,
                                 stop=C_row is None)
                if C_row is not None:
                    cmv = C_row[:].rearrange(
                        "p (c x) -> p c x", x=129)[:, c0:c0 + 4, 0:128]
                    nc.tensor.matmul(psY[:], ones_row_b[:], cmv,
                                     start=False, stop=True)
                m = (c0 // 4 + p) % 8
                if m < evac_dve:
                    # one DVE op for all 4 chunks: y = psY * R (R broadcast
                    # along the value columns with a stride-0 AP)
                    rb = Rp[:, c0:c0 + 4].unsqueeze(2).broadcast_to(
                        [128, 4, CH])
                    nc.vector.tensor_mul(
                        yall[:, c0 * CH:(c0 + 4) * CH].rearrange(
                            "p (c d) -> p c d", d=CH),
                        psY[:].rearrange("p (c d) -> p c d", d=CH), rb)
                else:
                    for i in range(4):
                        c = c0 + i
                        nc.scalar.activation(yall[:, c * CH:(c + 1) * CH],
                                             psY[:, i * CH:(i + 1) * CH],
                                             Act.Copy, scale=Rp[:, c:c + 1])
            nc.sync.dma_start(yd[p], yall[:])

        # all loads up front (y reuses v tiles, so everything fits),
        # then a 3-stage software pipeline in readiness order:
        # head1(p) | body(p-2) | head2(p-1)
        loads = [emit_loads(p) for p in range(pairs)]
        h1q, h2q = [], []
        def do_h1(p):
            if p < pairs:
                h1q.append((p, emit_head(p, *loads[p])))
        def do_h2():
            if h1q:
                p0, c0 = h1q.pop(0)
                h2q.append((p0, emit_head2(p0, *c0)))
        def do_b():
            if h2q:
                emit_body(*h2q.pop(0))
        orders = {
            0: lambda p: (do_h1(p), do_b(), do_h2()),
            1: lambda p: (do_b(), do_h2(), do_h1(p)),
            2: lambda p: (do_h1(p), do_h2(), do_b()),
            3: lambda p: (do_b(), do_h1(p), do_h2()),
            4: lambda p: (do_h2(), do_b(), do_h1(p)),
            5: lambda p: (do_h2(), do_h1(p), do_b()),
        }
        for p in range(pairs + 2):
            orders[order](p)

    nc.compile()
    return nc


def pack_qk(x):
    """[m, N, D] -> [m, D, N] fp16 (transposed)."""
    return np.ascontiguousarray(np.asarray(x, np.float16).transpose(0, 2, 1))


def pack_t(x):
    """[m, N, D] -> [m, 128 s, nch*128] fp16 t-major chunks."""
    m = x.shape[0]
    xp = np.asarray(x, np.float16).reshape(m, NCH, CH, D)
    return np.ascontiguousarray(
        xp.transpose(0, 2, 1, 3).reshape(m, CH, NCH * D))


def unpack_y(yp):
    """[m, 128, nch*128] -> [m, N, D] fp32."""
    m = yp.shape[0]
    yv = yp.reshape(m, CH, NCH, D).transpose(0, 2, 1, 3)
    return np.ascontiguousarray(yv.reshape(m, N, D).astype(np.float32))


_cached = {}


def _get_nc():
    if "nc" not in _cached:
        _cached["nc"] = build_nc()
    return _cached["nc"]


def run_on_hw(q, k, v, trace=False):
    """q,k,v: np [B,H,N,D] f32 -> (y [B,H,N,D], exec_time_ns or None)."""
    from concourse.bass_utils import run_bass_kernel_spmd

    nc = _get_nc()
    qp = pack_t(np.asarray(q, np.float32).reshape(B * H, N, D))
    kp = pack_t(np.asarray(k, np.float32).reshape(B * H, N, D))
    vp = pack_t(np.asarray(v, np.float32).reshape(B * H, N, D))
    in_maps = [
        {
            "q": qp[c * PAIRS:(c + 1) * PAIRS],
            "k": kp[c * PAIRS:(c + 1) * PAIRS],
            "v": vp[c * PAIRS:(c + 1) * PAIRS],
        }
        for c in range(NCORES)
    ]
    res = run_bass_kernel_spmd(nc, in_maps, list(range(NCORES)), trace=False)
    yp = np.concatenate([np.asarray(res.results[c]["y"]) for c in range(NCORES)],
                        axis=0)
    return unpack_y(yp).reshape(B, H, N, D), res.exec_time_ns


def kernel(q, k, v):
    y, _ = run_on_hw(q, k, v, trace=False)
    return y


# revision 5
# speedup vs baseline: 1.0582x; 1.0423x over previous
"""v2: carry-chain-free Aaren scan kernel (fp16 DMA, bf16 matmul operands).

Layouts (per (b,h) pair, CH=128, nch=32, no padding):
- q,k packed TRANSPOSED [128 d, 4096 t] fp16. prod = q*k (one DVE 4x mul);
  alpha chunk c = ones-matmul with stationary prod[:, chunk] and moving
  ones[128,1] -> psA[:, c] COLUMN (t-major [128, 32], quadrant-legal,
  ap_size=1 so nearly free on PE).
- v,y packed t-major [128 s, 32c*128] fp16.
- wv staging [128, 32*129] bf16: per chunk 128 scaled v-cols + col 128 =
  W column (denominator rides through all matmuls as a 129th column).

Math: running chunk max M_c stabilizer; W = exp(Ap - M) (fp32 for scale
ops, bf16 copy for matmuls; bf16 keeps e^-40-scale weights from flushing).
Cross-chunk state in closed form (no serial carry chain):
  S_c (values) = ones-matmul with stationary wv chunk -> psS_T[:, c] column;
  transposed once per pair to c-major; S^d col = ones-matmul of W_t.
  C = G' @ S_sb, G'[j,c] = exp(min(M_j-M_c,0)) * [j<c] (mask after exp).
  C rows flattened to partition-0 row C_row via one tiny SBUF->SBUF DMA
  (matmul moving operands must be quadrant-aligned).
Per chunk: psY = U2^T wv_c + rank-1 ones x C_row-slice; y = psY[:,0:128]
* R[:,c] with R = 1/(U2^T W_t + ones x CdT) once per pair.

Sharding: B*H = 64 pairs -> 8 pairs per core, no cross-core traffic.
"""

import sys

if "/opt/trn_rl_repo" not in sys.path:
    sys.path.insert(0, "/opt/trn_rl_repo")

import numpy as np

B, H, N, D = 4, 16, 4096, 128
NCORES = 8
PAIRS = B * H // NCORES  # 8 (b,h) pairs per core

CH = 128
NCH = N // CH       # 32 chunks
FW = N
WVW = NCH * (CH + 1)
NEG = -3.0e38
STORE_LAG = 3
LOOKAHEAD = 1
ALPHA_MODE = "hybrid"


def build_nc(pairs=PAIRS, n=N, order=0, evac_dve=6, presc_pool=16,
             alpha_pool=0,
             py_bufs=3, pg_bufs=4, skip_carry=False, skip_alpha=False,
             qk_bufs=4, wv_bufs=3, sc_bufs=4, sp_bufs=3):
    import concourse.tile as tile
    from concourse import bacc, mybir
    from contextlib import ExitStack

    f32 = mybir.dt.float32
    f16 = mybir.dt.float16
    bf16 = mybir.dt.bfloat16
    Alu = mybir.AluOpType
    Act = mybir.ActivationFunctionType
    X = mybir.AxisListType.X

    nch = n // CH
    fw = n
    wvw = nch * (CH + 1)

    nc = bacc.Bacc("TRN2", target_bir_lowering=False, debug=False)

    qd = nc.dram_tensor("q", [pairs, 128, fw], f16, kind="ExternalInput")
    kd = nc.dram_tensor("k", [pairs, 128, fw], f16, kind="ExternalInput")
    vd = nc.dram_tensor("v", [pairs, 128, fw], f16, kind="ExternalInput")
    yd = nc.dram_tensor("y", [pairs, 128, fw], f16, kind="ExternalOutput")

    with tile.TileContext(nc) as tc, ExitStack() as ctx:
        cpool = ctx.enter_context(tc.tile_pool(name="consts", bufs=1))
        spool = ctx.enter_context(tc.tile_pool(name="stats", bufs=sp_bufs))
        wpool = ctx.enter_context(tc.tile_pool(name="wts", bufs=4))
        rpool = ctx.enter_context(tc.tile_pool(name="rcps", bufs=3))
        qkpool = ctx.enter_context(tc.tile_pool(name="qk", bufs=qk_bufs))
        scpool = ctx.enter_context(tc.tile_pool(name="scr", bufs=sc_bufs))
        vpool = ctx.enter_context(tc.tile_pool(name="vv", bufs=8))
        wvpool = ctx.enter_context(tc.tile_pool(name="wv", bufs=wv_bufs))
        pypool = ctx.enter_context(tc.tile_pool(name="py", bufs=py_bufs, space="PSUM"))
        chpool = ctx.enter_context(tc.tile_pool(name="ch", bufs=pg_bufs, space="PSUM"))
        pbpool = ctx.enter_context(tc.tile_pool(name="pb", bufs=1, space="PSUM"))

        # ---- constants -------------------------------------------------
        iota_f = cpool.tile([128, 128], f32, tag="iotaf")
        nc.gpsimd.iota(iota_f[:], [[1, 128]], channel_multiplier=0,
                       allow_small_or_imprecise_dtypes=True)
        iota_p = cpool.tile([128, 1], f32, tag="iotap")
        nc.gpsimd.iota(iota_p[:], [[0, 1]], channel_multiplier=1,
                       allow_small_or_imprecise_dtypes=True)
        u2f = cpool.tile([128, 128], f32, tag="u2f")
        nc.vector.tensor_scalar(u2f[:], iota_f[:], iota_p[:], None, Alu.is_ge)
        u2h = cpool.tile([128, 128], bf16, tag="u2h")
        nc.vector.tensor_copy(u2h[:], u2f[:])
        ident = cpool.tile([128, 128], f32, tag="ident")
        nc.vector.tensor_scalar(ident[:], iota_f[:], iota_p[:], None,
                                Alu.is_equal)
        identb = cpool.tile([128, 128], bf16, tag="identb")
        nc.vector.tensor_copy(identb[:], ident[:])
        mltf = cpool.tile([32, 32], f32, tag="mltf")
        nc.vector.tensor_scalar(mltf[:], iota_f[0:32, 0:32], iota_p[0:32, :],
                                None, Alu.is_gt)
        mltb = cpool.tile([32, 32], bf16, tag="mltb")
        nc.vector.tensor_copy(mltb[:], mltf[:])
        ones_col_h = cpool.tile([128, 1], f16, tag="onescolh")
        nc.gpsimd.memset(ones_col_h[:], 1.0)
        ones_col_b = cpool.tile([128, 1], bf16, tag="onescolb")
        nc.gpsimd.memset(ones_col_b[:], 1.0)
        ones_row32 = cpool.tile([1, 32], f32, tag="onesrow32")
        nc.gpsimd.memset(ones_row32[:], 1.0)
        ones_row_f = cpool.tile([1, 128], f32, tag="onesrowf")
        nc.gpsimd.memset(ones_row_f[:], 1.0)
        ones_row_b = cpool.tile([1, 128], bf16, tag="onesrowb")
        nc.gpsimd.memset(ones_row_b[:], 1.0)

        def emit_loads(p):
            qall = qkpool.tile([128, fw], f16, tag="qa")
            kall = qkpool.tile([128, fw], f16, tag="ka")
            nc.sync.dma_start(qall[:], qd[p])
            nc.sync.dma_start(kall[:], kd[p])
            vall = vpool.tile([128, fw], f16, tag="va", name=f"va{p}")
            nc.sync.dma_start(vall[:], vd[p])
            return qall, kall, vall

        def emit_head(p, qall, kall, vall):

            # ---- alpha via fused q*k multiply-accumulate (DVE) ---------
            # scalar_tensor_tensor: out = (q bypass 1) * k, accum_out = per-
            # partition free-dim sum = alpha column for the chunk.
            Ap = spool.tile([128, nch], f32, tag="Ap")
            for c in range(0 if skip_alpha else nch):
                scr = scpool.tile([128, D], f16, tag="scr")
                eng = nc.gpsimd if c % 8 < alpha_pool else nc.vector
                eng.scalar_tensor_tensor(
                    scr[:], qall[:, c * CH:(c + 1) * CH], 1.0,
                    kall[:, c * CH:(c + 1) * CH],
                    op0=Alu.bypass, op1=Alu.mult,
                    accum_out=Ap[:, c:c + 1])

            # ---- running chunk max M (regions of one psum bank tile) ---
            chain = chpool.tile([128, 512], f32, tag="ch", name=f"chain{p}")
            psT2 = chain
            nc.tensor.transpose(psT2[0:32, 256:384], Ap[:], ident[:])
            mu = spool.tile([32, 1], f32, tag="mu")
            nc.vector.tensor_reduce(mu[:], psT2[0:32, 256:384],
                                    axis=X, op=Alu.max)
            psT = chain
            nc.tensor.transpose(psT[0:1, 448:480], mu[:], ident[0:32, 0:32])
            mrow = spool.tile([1, 32], f32, tag="mrow")
            nc.vector.tensor_copy(mrow[:], psT[0:1, 448:480])
            Mrow = spool.tile([1, 32], f32, tag="Mrow")
            nc.vector.tensor_tensor_scan(Mrow[:], mrow[:], mrow[:],
                                         initial=NEG, op0=Alu.max, op1=Alu.max)
            negM = spool.tile([1, 32], f32, tag="negM")
            nc.vector.tensor_scalar_mul(negM[:], Mrow[:], -1.0)

            # ---- G'[j,c] = exp(min(M_j - M_c, 0)) * [j < c] ------------
            psG = chain
            nc.tensor.matmul(psG[32:64, 384:416], Mrow[:], ones_row32[:],
                             start=True, stop=False)
            nc.tensor.matmul(psG[32:64, 384:416], ones_row32[:], negM[:],
                             start=False, stop=True)
            gclip = spool.tile([32, 32], f32, tag="gclip")
            nc.vector.tensor_scalar_min(gclip[:], psG[32:64, 384:416], 0.0)
            gexp = spool.tile([32, 32], bf16, tag="gexp")
            nc.scalar.activation(gexp[:], gclip[:], Act.Exp)
            Gh = spool.tile([32, 32], bf16, tag="Gh")
            nc.vector.tensor_mul(Gh[:], gexp[:], mltb[:])

            # ---- weights W = exp(Ap - M): f32 + bf16 copies ------------
            psB = chain
            nc.tensor.matmul(psB[0:128, 416:448], ones_row_f[:], Mrow[:],
                             start=True, stop=True)
            wsub = spool.tile([128, 32], f32, tag="wsub")
            nc.vector.tensor_sub(wsub[:], Ap[:], psB[0:128, 416:448])
            W_f = wpool.tile([128, 32], f32, tag="W_f")
            nc.scalar.activation(W_f[:], wsub[:], Act.Exp)
            return vall, Gh, W_f, chain

        def emit_head2(p, vall, Gh, W_f, chain):
            # ---- prescale v into wv (129-col blocks) -------------------
            wv = wvpool.tile([128, wvw], bf16, tag="wv")
            for c in range(nch):
                dst = wv[:, c * 129:c * 129 + 128]
                src = vall[:, c * CH:(c + 1) * CH]
                if (c * 32) // 32 % 32 < presc_pool if False else c % 32 < presc_pool:
                    nc.gpsimd.tensor_scalar_mul(dst, src, W_f[:, c:c + 1])
                else:
                    nc.scalar.activation(dst, src, Act.Copy,
                                         scale=W_f[:, c:c + 1])
            wv_d = wv[:].rearrange("p (c x) -> p c x", x=129)[:, :, 128:129]
            nc.vector.tensor_copy(
                wv_d, W_f[:].rearrange("p (c x) -> p c x", x=1))

            if skip_carry:
                Rp = rpool.tile([128, 32], f32, tag="Rp")
                nc.gpsimd.memset(Rp[:], 1.0)
                return wv, Rp, None, vall
            # ---- chunk sums S via PE column matmuls + transpose --------
            psS_T = chain
            for c in range(nch):
                nc.tensor.matmul(psS_T[0:128, c:c + 1],
                                 wv[:, c * 129:c * 129 + 128],
                                 ones_col_b[:], start=True, stop=True)
            psSd = psS_T
            nc.tensor.matmul(psSd[0:32, 32:33], wv_d.squeeze(2),
                             ones_col_b[:], start=True, stop=True)
            S_T_sb = spool.tile([128, 32], bf16, tag="S_T_sb")
            nc.scalar.copy(S_T_sb[:], psS_T[0:128, 0:32])
            psS2 = pbpool.tile([128, 129], bf16, tag="pb", name=f"psS2{p}")
            nc.tensor.transpose(psS2[0:32, 0:128], S_T_sb[:], identb[:])
            S_sb = spool.tile([32, 129], bf16, tag="S_sb")
            nc.scalar.copy(S_sb[0:32, 0:128], psS2[0:32, 0:128])
            nc.scalar.copy(S_sb[0:32, 128:129], psSd[0:32, 32:33])

            # ---- carries C = G' @ S ------------------------------------
            psN = chain
            nc.tensor.matmul(psN[0:32, 64:193], Gh[:], S_sb[:],
                             start=True, stop=True)
            C_sb = spool.tile([32, 129], bf16, tag="C_sb")
            nc.scalar.copy(C_sb[:], psN[0:32, 64:193])
            # Flatten carries to a partition-0 row (rank-1 carry matmuls need
            # quadrant-aligned moving operands).
            C_row = spool.tile([1, nch * 129], bf16, tag="C_row")
            nc.sync.dma_start(C_row[:], C_sb[:])

            # ---- denominators (reciprocal deferred to the body) --------
            CdT = spool.tile([1, 32], bf16, tag="CdT")
            nc.vector.tensor_copy(
                CdT[:],
                C_row[:].rearrange("p (c x) -> p c x", x=129)[:, :, 128])
            psD = chain
            nc.tensor.matmul(psD[0:128, 200:232], u2h[:], wv_d.squeeze(2),
                             start=True, stop=False)
            nc.tensor.matmul(psD[0:128, 200:232], ones_row_b[:], CdT[:],
                             start=False, stop=True)
            Rp = rpool.tile([128, 32], f32, tag="Rp")
            nc.vector.reciprocal(Rp[:], psD[0:128, 200:232])
            return wv, Rp, C_row, vall

        def emit_body(p, ctx2):
            wv, Rp, C_row, vall = ctx2
            # ---- main chunk matmuls (4 chunks per matmul; moving operand
            # skips the d-cols via a strided AP) + multi-chunk evacs ------
            # y is written in place over v (v is dead after the prescale)
            yall = vall
            for c0 in range(0, nch, 4):
                psY = pypool.tile([128, 512], f32, tag="psY")
                mv = wv[:].rearrange("p (c x) -> p c x",
                                     x=129)[:, c0:c0 + 4, 0:128]
                # tri part depends only on wv (can run early); the rank-1
                # carry gates on C_row, so only it + the evacs wait on C
                nc.tensor.matmul(psY[:], u2h[:], mv, start=True,
                                 stop=C_row is None)
                if C_row is not None:
                    cmv = C_row[:].rearrange(
                        "p (c x) -> p c x", x=129)[:, c0:c0 + 4, 0:128]
                    nc.tensor.matmul(psY[:], ones_row_b[:], cmv,
                                     start=False, stop=True)
                m = (c0 // 4 + p) % 8
                if m < evac_dve:
                    # one DVE op for all 4 chunks: y = psY * R (R broadcast
                    # along the value columns with a stride-0 AP)
                    rb = Rp[:, c0:c0 + 4].unsqueeze(2).broadcast_to(
                        [128, 4, CH])
                    nc.vector.tensor_mul(
                        yall[:, c0 * CH:(c0 + 4) * CH].rearrange(
                            "p (c d) -> p c d", d=CH),
                        psY[:].rearrange("p (c d) -> p c d", d=CH), rb)
                else:
                    for i in range(4):
                        c = c0 + i
                        nc.scalar.activation(yall[:, c * CH:(c + 1) * CH],
                                             psY[:, i * CH:(i + 1) * CH],
                                             Act.Copy, scale=Rp[:, c:c + 1])
            nc.sync.dma_start(yd[p], yall[:])

        # all loads up front (y reuses v tiles, so everything fits),
        # then a 3-stage software pipeline in readiness order:
        # head1(p) | body(p-2) | head2(p-1)
        loads = [emit_loads(p) for p in range(pairs)]
        h1q, h2q = [], []
        def do_h1(p):
            if p < pairs:
                h1q.append((p, emit_head(p, *loads[p])))
        def do_h2():
            if h1q:
                p0, c0 = h1q.pop(0)
                h2q.append((p0, emit_head2(p0, *c0)))
        def do_b():
            if h2q:
                emit_body(*h2q.pop(0))
        orders = {
            0: lambda p: (do_h1(p), do_b(), do_h2()),
            1: lambda p: (do_b(), do_h2(), do_h1(p)),
            2: lambda p: (do_h1(p), do_h2(), do_b()),
            3: lambda p: (do_b(), do_h1(p), do_h2()),
            4: lambda p: (do_h2(), do_b(), do_h1(p)),
            5: lambda p: (do_h2(), do_h1(p), do_b()),
        }
        for p in range(pairs + 2):
            orders[order](p)

    nc.compile()
    return nc


def pack_qk(x):
    """[m, N, D] -> [m, D, N] fp16 (transposed)."""
    return np.ascontiguousarray(np.asarray(x, np.float16).transpose(0, 2, 1))


def pack_t(x):
    """[m, N, D] -> [m, 128 s, nch*128] fp16 t-major chunks."""
    m = x.shape[0]
    xp = np.asarray(x, np.float16).reshape(m, NCH, CH, D)
    return np.ascontiguousarray(
        xp.transpose(0, 2, 1, 3).reshape(m, CH, NCH * D))


def unpack_y(yp):
    """[m, 128, nch*128] -> [m, N, D] fp32."""
    m = yp.shape[0]
    yv = yp.reshape(m, CH, NCH, D).transpose(0, 2, 1, 3)
    return np.ascontiguousarray(yv.reshape(m, N, D).astype(np.float32))


_cached = {}


def _get_nc():
    if "nc" not in _cached:
        _cached["nc"] = build_nc()
    return _cached["nc"]


def run_on_hw(q, k, v, trace=False):
    """q,k,v: np [B,H,N,D] f32 -> (y [B,H,N,D], exec_time_ns or None)."""
    from concourse.bass_utils import run_bass_kernel_spmd

    nc = _get_nc()
    qp = pack_t(np.asarray(q, np.float32).reshape(B * H, N, D))
    kp = pack_t(np.asarray(k, np.float32).reshape(B * H, N, D))
    vp = pack_t(np.asarray(v, np.float32).reshape(B * H, N, D))
    in_maps = [
        {
            "q": qp[c * PAIRS:(c + 1) * PAIRS],
            "k": kp[c * PAIRS:(c + 1) * PAIRS],
            "v": vp[c * PAIRS:(c + 1) * PAIRS],
        }
        for c in range(NCORES)
    ]
    res = run_bass_kernel_spmd(nc, in_maps, list(range(NCORES)), trace=False)
    yp = np.concatenate([np.asarray(res.results[c]["y"]) for c in range(NCORES)],
                        axis=0)
    return unpack_y(yp).reshape(B, H, N, D), res.exec_time_ns


def kernel(q, k, v):
    y, _ = run_on_hw(q, k, v, trace=False)
    return y


# revision 6
# speedup vs baseline: 1.0680x; 1.0093x over previous
"""v2: carry-chain-free Aaren scan kernel (fp16 DMA, bf16 matmul operands).

Layouts (per (b,h) pair, CH=128, nch=32, no padding):
- q,k packed TRANSPOSED [128 d, 4096 t] fp16. prod = q*k (one DVE 4x mul);
  alpha chunk c = ones-matmul with stationary prod[:, chunk] and moving
  ones[128,1] -> psA[:, c] COLUMN (t-major [128, 32], quadrant-legal,
  ap_size=1 so nearly free on PE).
- v,y packed t-major [128 s, 32c*128] fp16.
- wv staging [128, 32*129] bf16: per chunk 128 scaled v-cols + col 128 =
  W column (denominator rides through all matmuls as a 129th column).

Math: running chunk max M_c stabilizer; W = exp(Ap - M) (fp32 for scale
ops, bf16 copy for matmuls; bf16 keeps e^-40-scale weights from flushing).
Cross-chunk state in closed form (no serial carry chain):
  S_c (values) = ones-matmul with stationary wv chunk -> psS_T[:, c] column;
  transposed once per pair to c-major; S^d col = ones-matmul of W_t.
  C = G' @ S_sb, G'[j,c] = exp(min(M_j-M_c,0)) * [j<c] (mask after exp).
  C rows flattened to partition-0 row C_row via one tiny SBUF->SBUF DMA
  (matmul moving operands must be quadrant-aligned).
Per chunk: psY = U2^T wv_c + rank-1 ones x C_row-slice; y = psY[:,0:128]
* R[:,c] with R = 1/(U2^T W_t + ones x CdT) once per pair.

Sharding: B*H = 64 pairs -> 8 pairs per core, no cross-core traffic.
"""

import sys

if "/opt/trn_rl_repo" not in sys.path:
    sys.path.insert(0, "/opt/trn_rl_repo")

import numpy as np

B, H, N, D = 4, 16, 4096, 128
NCORES = 8
PAIRS = B * H // NCORES  # 8 (b,h) pairs per core

CH = 128
NCH = N // CH       # 32 chunks
FW = N
WVW = NCH * (CH + 1)
NEG = -3.0e38
STORE_LAG = 3
LOOKAHEAD = 1
ALPHA_MODE = "hybrid"


def build_nc(pairs=PAIRS, n=N, order=0, evac_dve=7, presc_pool=10,
             alpha_pool=0,
             py_bufs=3, pg_bufs=4, skip_carry=False, skip_alpha=False,
             qk_bufs=3, wv_bufs=4, sc_bufs=4, sp_bufs=3):
    import concourse.tile as tile
    from concourse import bacc, mybir
    from contextlib import ExitStack

    f32 = mybir.dt.float32
    f16 = mybir.dt.float16
    bf16 = mybir.dt.bfloat16
    Alu = mybir.AluOpType
    Act = mybir.ActivationFunctionType
    X = mybir.AxisListType.X

    nch = n // CH
    fw = n
    wvw = nch * (CH + 1)

    nc = bacc.Bacc("TRN2", target_bir_lowering=False, debug=False)

    qd = nc.dram_tensor("q", [pairs, 128, fw], f16, kind="ExternalInput")
    kd = nc.dram_tensor("k", [pairs, 128, fw], f16, kind="ExternalInput")
    vd = nc.dram_tensor("v", [pairs, 128, fw], f16, kind="ExternalInput")
    yd = nc.dram_tensor("y", [pairs, 128, fw], f16, kind="ExternalOutput")

    with tile.TileContext(nc) as tc, ExitStack() as ctx:
        cpool = ctx.enter_context(tc.tile_pool(name="consts", bufs=1))
        spool = ctx.enter_context(tc.tile_pool(name="stats", bufs=sp_bufs))
        wpool = ctx.enter_context(tc.tile_pool(name="wts", bufs=4))
        rpool = ctx.enter_context(tc.tile_pool(name="rcps", bufs=3))
        qkpool = ctx.enter_context(tc.tile_pool(name="qk", bufs=qk_bufs))
        scpool = ctx.enter_context(tc.tile_pool(name="scr", bufs=sc_bufs))
        vpool = ctx.enter_context(tc.tile_pool(name="vv", bufs=8))
        wvpool = ctx.enter_context(tc.tile_pool(name="wv", bufs=wv_bufs))
        pypool = ctx.enter_context(tc.tile_pool(name="py", bufs=py_bufs, space="PSUM"))
        chpool = ctx.enter_context(tc.tile_pool(name="ch", bufs=pg_bufs, space="PSUM"))
        pbpool = ctx.enter_context(tc.tile_pool(name="pb", bufs=1, space="PSUM"))

        # ---- constants -------------------------------------------------
        iota_f = cpool.tile([128, 128], f32, tag="iotaf")
        nc.gpsimd.iota(iota_f[:], [[1, 128]], channel_multiplier=0,
                       allow_small_or_imprecise_dtypes=True)
        iota_p = cpool.tile([128, 1], f32, tag="iotap")
        nc.gpsimd.iota(iota_p[:], [[0, 1]], channel_multiplier=1,
                       allow_small_or_imprecise_dtypes=True)
        u2f = cpool.tile([128, 128], f32, tag="u2f")
        nc.vector.tensor_scalar(u2f[:], iota_f[:], iota_p[:], None, Alu.is_ge)
        u2h = cpool.tile([128, 128], bf16, tag="u2h")
        nc.vector.tensor_copy(u2h[:], u2f[:])
        ident = cpool.tile([128, 128], f32, tag="ident")
        nc.vector.tensor_scalar(ident[:], iota_f[:], iota_p[:], None,
                                Alu.is_equal)
        identb = cpool.tile([128, 128], bf16, tag="identb")
        nc.vector.tensor_copy(identb[:], ident[:])
        mltf = cpool.tile([32, 32], f32, tag="mltf")
        nc.vector.tensor_scalar(mltf[:], iota_f[0:32, 0:32], iota_p[0:32, :],
                                None, Alu.is_gt)
        mltb = cpool.tile([32, 32], bf16, tag="mltb")
        nc.vector.tensor_copy(mltb[:], mltf[:])
        ones_col_h = cpool.tile([128, 1], f16, tag="onescolh")
        nc.gpsimd.memset(ones_col_h[:], 1.0)
        ones_col_b = cpool.tile([128, 1], bf16, tag="onescolb")
        nc.gpsimd.memset(ones_col_b[:], 1.0)
        ones_row32 = cpool.tile([1, 32], f32, tag="onesrow32")
        nc.gpsimd.memset(ones_row32[:], 1.0)
        ones_row_f = cpool.tile([1, 128], f32, tag="onesrowf")
        nc.gpsimd.memset(ones_row_f[:], 1.0)
        ones_row_b = cpool.tile([1, 128], bf16, tag="onesrowb")
        nc.gpsimd.memset(ones_row_b[:], 1.0)

        def emit_loads(p):
            qall = qkpool.tile([128, fw], f16, tag="qa")
            kall = qkpool.tile([128, fw], f16, tag="ka")
            nc.sync.dma_start(qall[:], qd[p])
            nc.sync.dma_start(kall[:], kd[p])
            vall = vpool.tile([128, fw], f16, tag="va", name=f"va{p}")
            nc.sync.dma_start(vall[:], vd[p])
            return qall, kall, vall

        def emit_head(p, qall, kall, vall):

            # ---- alpha via fused q*k multiply-accumulate (DVE) ---------
            # scalar_tensor_tensor: out = (q bypass 1) * k, accum_out = per-
            # partition free-dim sum = alpha column for the chunk.
            Ap = spool.tile([128, nch], f32, tag="Ap")
            for c in range(0 if skip_alpha else nch):
                scr = scpool.tile([128, D], f16, tag="scr")
                eng = nc.gpsimd if c % 8 < alpha_pool else nc.vector
                eng.scalar_tensor_tensor(
                    scr[:], qall[:, c * CH:(c + 1) * CH], 1.0,
                    kall[:, c * CH:(c + 1) * CH],
                    op0=Alu.bypass, op1=Alu.mult,
                    accum_out=Ap[:, c:c + 1])

            # ---- running chunk max M (regions of one psum bank tile) ---
            chain = chpool.tile([128, 512], f32, tag="ch", name=f"chain{p}")
            psT2 = chain
            nc.tensor.transpose(psT2[0:32, 256:384], Ap[:], ident[:])
            mu = spool.tile([32, 1], f32, tag="mu")
            nc.vector.tensor_reduce(mu[:], psT2[0:32, 256:384],
                                    axis=X, op=Alu.max)
            psT = chain
            nc.tensor.transpose(psT[0:1, 448:480], mu[:], ident[0:32, 0:32])
            mrow = spool.tile([1, 32], f32, tag="mrow")
            nc.vector.tensor_copy(mrow[:], psT[0:1, 448:480])
            Mrow = spool.tile([1, 32], f32, tag="Mrow")
            nc.vector.tensor_tensor_scan(Mrow[:], mrow[:], mrow[:],
                                         initial=NEG, op0=Alu.max, op1=Alu.max)
            negM = spool.tile([1, 32], f32, tag="negM")
            nc.vector.tensor_scalar_mul(negM[:], Mrow[:], -1.0)

            # ---- G'[j,c] = exp(min(M_j - M_c, 0)) * [j < c] ------------
            psG = chain
            nc.tensor.matmul(psG[32:64, 384:416], Mrow[:], ones_row32[:],
                             start=True, stop=False)
            nc.tensor.matmul(psG[32:64, 384:416], ones_row32[:], negM[:],
                             start=False, stop=True)
            gclip = spool.tile([32, 32], f32, tag="gclip")
            nc.vector.tensor_scalar_min(gclip[:], psG[32:64, 384:416], 0.0)
            gexp = spool.tile([32, 32], bf16, tag="gexp")
            nc.scalar.activation(gexp[:], gclip[:], Act.Exp)
            Gh = spool.tile([32, 32], bf16, tag="Gh")
            nc.vector.tensor_mul(Gh[:], gexp[:], mltb[:])

            # ---- weights W = exp(Ap - M): f32 + bf16 copies ------------
            psB = chain
            nc.tensor.matmul(psB[0:128, 416:448], ones_row_f[:], Mrow[:],
                             start=True, stop=True)
            wsub = spool.tile([128, 32], f32, tag="wsub")
            nc.vector.tensor_sub(wsub[:], Ap[:], psB[0:128, 416:448])
            W_f = wpool.tile([128, 32], f32, tag="W_f")
            nc.scalar.activation(W_f[:], wsub[:], Act.Exp)
            return vall, Gh, W_f, chain

        def emit_head2(p, vall, Gh, W_f, chain):
            # ---- prescale v into wv (129-col blocks) -------------------
            wv = wvpool.tile([128, wvw], bf16, tag="wv")
            for c in range(nch):
                dst = wv[:, c * 129:c * 129 + 128]
                src = vall[:, c * CH:(c + 1) * CH]
                if (c * 32) // 32 % 32 < presc_pool if False else c % 32 < presc_pool:
                    nc.gpsimd.tensor_scalar_mul(dst, src, W_f[:, c:c + 1])
                else:
                    nc.scalar.activation(dst, src, Act.Copy,
                                         scale=W_f[:, c:c + 1])
            wv_d = wv[:].rearrange("p (c x) -> p c x", x=129)[:, :, 128:129]
            nc.vector.tensor_copy(
                wv_d, W_f[:].rearrange("p (c x) -> p c x", x=1))

            if skip_carry:
                Rp = rpool.tile([128, 32], f32, tag="Rp")
                nc.gpsimd.memset(Rp[:], 1.0)
                return wv, Rp, None, vall
            # ---- chunk sums S via PE column matmuls + transpose --------
            psS_T = chain
            for c in range(nch):
                nc.tensor.matmul(psS_T[0:128, c:c + 1],
                                 wv[:, c * 129:c * 129 + 128],
                                 ones_col_b[:], start=True, stop=True)
            psSd = psS_T
            nc.tensor.matmul(psSd[0:32, 32:33], wv_d.squeeze(2),
                             ones_col_b[:], start=True, stop=True)
            S_T_sb = spool.tile([128, 32], bf16, tag="S_T_sb")
            nc.scalar.copy(S_T_sb[:], psS_T[0:128, 0:32])
            psS2 = pbpool.tile([128, 129], bf16, tag="pb", name=f"psS2{p}")
            nc.tensor.transpose(psS2[0:32, 0:128], S_T_sb[:], identb[:])
            S_sb = spool.tile([32, 129], bf16, tag="S_sb")
            nc.scalar.copy(S_sb[0:32, 0:128], psS2[0:32, 0:128])
            nc.scalar.copy(S_sb[0:32, 128:129], psSd[0:32, 32:33])

            # ---- carries C = G' @ S ------------------------------------
            psN = chain
            nc.tensor.matmul(psN[0:32, 64:193], Gh[:], S_sb[:],
                             start=True, stop=True)
            C_sb = spool.tile([32, 129], bf16, tag="C_sb")
            nc.scalar.copy(C_sb[:], psN[0:32, 64:193])
            # Flatten carries to a partition-0 row (rank-1 carry matmuls need
            # quadrant-aligned moving operands).
            C_row = spool.tile([1, nch * 129], bf16, tag="C_row")
            nc.sync.dma_start(C_row[:], C_sb[:])

            # ---- denominators (reciprocal deferred to the body) --------
            CdT = spool.tile([1, 32], bf16, tag="CdT")
            nc.vector.tensor_copy(
                CdT[:],
                C_row[:].rearrange("p (c x) -> p c x", x=129)[:, :, 128])
            psD = chain
            nc.tensor.matmul(psD[0:128, 200:232], u2h[:], wv_d.squeeze(2),
                             start=True, stop=False)
            nc.tensor.matmul(psD[0:128, 200:232], ones_row_b[:], CdT[:],
                             start=False, stop=True)
            Rp = rpool.tile([128, 32], f32, tag="Rp")
            nc.vector.reciprocal(Rp[:], psD[0:128, 200:232])
            return wv, Rp, C_row, vall

        def emit_body(p, ctx2):
            wv, Rp, C_row, vall = ctx2
            # ---- main chunk matmuls (4 chunks per matmul; moving operand
            # skips the d-cols via a strided AP) + multi-chunk evacs ------
            # y is written in place over v (v is dead after the prescale)
            yall = vall
            for c0 in range(0, nch, 4):
                psY = pypool.tile([128, 512], f32, tag="psY")
                mv = wv[:].rearrange("p (c x) -> p c x",
                                     x=129)[:, c0:c0 + 4, 0:128]
                # tri part depends only on wv (can run early); the rank-1
                # carry gates on C_row, so only it + the evacs wait on C
                nc.tensor.matmul(psY[:], u2h[:], mv, start=True,
                                 stop=C_row is None)
                if C_row is not None:
                    cmv = C_row[:].rearrange(
                        "p (c x) -> p c x", x=129)[:, c0:c0 + 4, 0:128]
                    nc.tensor.matmul(psY[:], ones_row_b[:], cmv,
                                     start=False, stop=True)
                m = (c0 // 4 + p) % 8
                if m < evac_dve:
                    # one DVE op for all 4 chunks: y = psY * R (R broadcast
                    # along the value columns with a stride-0 AP)
                    rb = Rp[:, c0:c0 + 4].unsqueeze(2).broadcast_to(
                        [128, 4, CH])
                    nc.vector.tensor_mul(
                        yall[:, c0 * CH:(c0 + 4) * CH].rearrange(
                            "p (c d) -> p c d", d=CH),
                        psY[:].rearrange("p (c d) -> p c d", d=CH), rb)
                else:
                    for i in range(4):
                        c = c0 + i
                        nc.scalar.activation(yall[:, c * CH:(c + 1) * CH],
                                             psY[:, i * CH:(i + 1) * CH],
                                             Act.Copy, scale=Rp[:, c:c + 1])
            nc.sync.dma_start(yd[p], yall[:])

        # all loads up front (y reuses v tiles, so everything fits),
        # then a 3-stage software pipeline in readiness order:
        # head1(p) | body(p-2) | head2(p-1)
        loads = [emit_loads(p) for p in range(pairs)]
        h1q, h2q = [], []
        def do_h1(p):
            if p < pairs:
                h1q.append((p, emit_head(p, *loads[p])))
        def do_h2():
            if h1q:
                p0, c0 = h1q.pop(0)
                h2q.append((p0, emit_head2(p0, *c0)))
        def do_b():
            if h2q:
                emit_body(*h2q.pop(0))
        orders = {
            0: lambda p: (do_h1(p), do_b(), do_h2()),
            1: lambda p: (do_b(), do_h2(), do_h1(p)),
            2: lambda p: (do_h1(p), do_h2(), do_b()),
            3: lambda p: (do_b(), do_h1(p), do_h2()),
            4: lambda p: (do_h2(), do_b(), do_h1(p)),
            5: lambda p: (do_h2(), do_h1(p), do_b()),
        }
        for p in range(pairs + 2):
            orders[order](p)

    nc.compile()
    return nc


def pack_qk(x):
    """[m, N, D] -> [m, D, N] fp16 (transposed)."""
    return np.ascontiguousarray(np.asarray(x, np.float16).transpose(0, 2, 1))


def pack_t(x):
    """[m, N, D] -> [m, 128 s, nch*128] fp16 t-major chunks."""
    m = x.shape[0]
    xp = np.asarray(x, np.float16).reshape(m, NCH, CH, D)
    return np.ascontiguousarray(
        xp.transpose(0, 2, 1, 3).reshape(m, CH, NCH * D))


def unpack_y(yp):
    """[m, 128, nch*128] -> [m, N, D] fp32."""
    m = yp.shape[0]
    yv = yp.reshape(m, CH, NCH, D).transpose(0, 2, 1, 3)
    return np.ascontiguousarray(yv.reshape(m, N, D).astype(np.float32))


_cached = {}


def _get_nc():
    if "nc" not in _cached:
        _cached["nc"] = build_nc()
    return _cached["nc"]


def run_on_hw(q, k, v, trace=False):
    """q,k,v: np [B,H,N,D] f32 -> (y [B,H,N,D], exec_time_ns or None)."""
    from concourse.bass_utils import run_bass_kernel_spmd

    nc = _get_nc()
    qp = pack_t(np.asarray(q, np.float32).reshape(B * H, N, D))
    kp = pack_t(np.asarray(k, np.float32).reshape(B * H, N, D))
    vp = pack_t(np.asarray(v, np.float32).reshape(B * H, N, D))
    in_maps = [
        {
            "q": qp[c * PAIRS:(c + 1) * PAIRS],
            "k": kp[c * PAIRS:(c + 1) * PAIRS],
            "v": vp[c * PAIRS:(c + 1) * PAIRS],
        }
        for c in range(NCORES)
    ]
    res = run_bass_kernel_spmd(nc, in_maps, list(range(NCORES)), trace=False)
    yp = np.concatenate([np.asarray(res.results[c]["y"]) for c in range(NCORES)],
                        axis=0)
    return unpack_y(yp).reshape(B, H, N, D), res.exec_time_ns


def kernel(q, k, v):
    y, _ = run_on_hw(q, k, v, trace=False)
    return y
